# revision 9
# baseline (speedup 1.0000x reference)
"""Causal self-attention (sparse column mask) on 8 Trainium2 NeuronCores.

Problem: B=8, T=1024, C=512, 8 heads (hd=64).
  q/k/v = x @ W{q,k,v}.T + b;  att = softmax(mask(q k^T / 8));  y = att v
  out = y @ Wp.T + bp
Mask: causal lower-triangle, minus every column j with j % 25 == 24.

Sharding: pure data-parallel over batch — core b computes batch element b.

Per-core kernel design (all matmul operands fp16, PSUM accumulation f32):
  - Host pre-transposes x[b] -> xT [C, T]; q/k weights are packed per
    head-pair m ([P, KT*128], contiguous 1KB DMA lines) so head-pair 0's
    projections complete first and attention starts ~3 us earlier.
  - Inputs stream chunked + prioritized over three DMA queues (sync,
    gpsimd, scalar); the tensor engine issues none so matmuls start ASAP.
  - q/k projections are phase-split by T-half: half0 feeds attention ic=0
    while half1 still streams in. q bias added during PSUM evacuation
    (DVE tensor_scalar); k bias dropped (softmax shift invariance); v bias
    folded into the output bias on host (bp' = Wp @ bv + bp).
  - Attention per query chunk ic (512 wide):
      QK (64x128 row-tiled): per head-pair p, key tile J: two K=64 matmuls
      -> S^T in PSUM; one ACT exp (scale=1/8, per-partition bias -30 on
      j%25==24 columns) -> fp16 SBUF; causal diagonal zeroed by one fp16
      multiply with a broadcast lower-triangle tile on DVE.
      AV (128x64 col-tiled): accumulate y'^T and replicated denominators
      (ones-weight matmuls) over J; rden via approx reciprocal; one
      tensor_tensor multiply PSUM->SBUF fp16.
  - The emission weaves qk J-units (which feed the ACT exp pipeline, the
    per-element bottleneck) with budget-tracked PE filler (v/out/half1
    projections, AV phases) so ACT never starves and PE never idles.
  - fp16 output; the last av phase is column-split so rows 4..7 finish
    pipelined; their PSUM comes from the st pool (idle by then).
"""

import numpy as np

B, T, C = 8, 1024, 512
H = 8
HD = C // H
P = 128
JD = 25  # joined dim; column j masked when j % 25 == 24
N_CORES = 8
NEG = -30.0  # added post-scale; exp(-30) flushes to 0 in fp16

_CACHE = {}


def _build():
    import concourse.bass as bass
    import concourse.mybir as mybir
    import concourse.tile as tile
    from concourse import bacc

    f16 = mybir.dt.float16
    f32 = mybir.dt.float32
    AF = mybir.ActivationFunctionType
    ALU = mybir.AluOpType

    nc = bacc.Bacc("TRN2", target_bir_lowering=False, debug=False)

    KT = C // P  # 4 c_in tiles
    MT = C // P  # 4 c_out tiles (= head pairs)
    RT = T // P  # 8 t tiles
    HT = 512  # half of T

    xT = nc.dram_tensor("xT", [C, T], f16, kind="ExternalInput").ap()
    wqm_d = [
        nc.dram_tensor(f"wqm{m}", [P, KT * P], f16, kind="ExternalInput").ap()
        for m in range(MT)
    ]
    wkm_d = [
        nc.dram_tensor(f"wkm{m}", [P, KT * P], f16, kind="ExternalInput").ap()
        for m in range(MT)
    ]
    wvT = nc.dram_tensor("wvT", [C, C], f16, kind="ExternalInput").ap()
    wpT = nc.dram_tensor("wpT", [C, C], f16, kind="ExternalInput").ap()
    # combo16: tri [:,0:128] | ones64 [:,128:192]; combo32: cmask [:,0:8] | bq [:,8:12]
    combo16 = nc.dram_tensor("combo16", [P, P + HD], f16, kind="ExternalInput").ap()
    combo32 = nc.dram_tensor("combo32", [P, T // P + MT], f32, kind="ExternalInput").ap()
    bppb = nc.dram_tensor("bppb", [P, C], f16, kind="ExternalInput").ap()
    out = nc.dram_tensor("out", [T, C], f16, kind="ExternalOutput").ap()

    with tile.TileContext(nc) as tc:
        with (
            tc.tile_pool(name="const", bufs=1) as const,
            tc.tile_pool(name="persist", bufs=1) as persist,
            tc.tile_pool(name="es", bufs=24) as es_pool,
            tc.tile_pool(name="rden", bufs=4) as rden_pool,
            tc.tile_pool(name="ot", bufs=4) as ot_pool,
            tc.tile_pool(name="pbig", bufs=2, space="PSUM") as pbig,
            tc.tile_pool(name="pA", bufs=2, space="PSUM") as pA,
            tc.tile_pool(name="pB", bufs=2, space="PSUM") as pB,
        ):
            # ---- chunked, prioritized input loads over three DMA queues ----
            def load(shape, dtype, src, tag, eng):
                t = const.tile(shape, dtype, name=tag, tag=tag)
                eng.dma_start(out=t, in_=src)
                return t

            r3 = lambda a: a.rearrange("(a p) n -> p a n", p=P)  # noqa: E731
            xTr, wvr, wpr = map(r3, (xT, wvT, wpT))

            xh = [[None] * KT for _ in range(2)]
            wqm_s, wkm_s = [None] * MT, [None] * MT
            wv_c, wp_c = [None] * KT, [None] * MT
            # strict per-queue priority; sync/gpsimd carry the bulk, scalar
            # (slower queue) takes the small combos + last-pair weights
            cm32 = load([P, T // P + MT], f32, combo32, "c32", nc.scalar)
            cm16 = load([P, P + HD], f16, combo16, "c16", nc.scalar)
            xh[0][0] = load([P, 1, HT], f16, xTr[:, 0:1, 0:HT], "xh00", nc.sync)
            xh[0][1] = load([P, 1, HT], f16, xTr[:, 1:2, 0:HT], "xh01", nc.gpsimd)
            wqm_s[0] = load([P, KT * P], f16, wqm_d[0], "wqm0", nc.sync)
            wkm_s[0] = load([P, KT * P], f16, wkm_d[0], "wkm0", nc.gpsimd)
            xh[0][2] = load([P, 1, HT], f16, xTr[:, 2:3, 0:HT], "xh02", nc.sync)
            xh[0][3] = load([P, 1, HT], f16, xTr[:, 3:4, 0:HT], "xh03", nc.gpsimd)
            wqm_s[3] = load([P, KT * P], f16, wqm_d[3], "wqm3", nc.scalar)
            wqm_s[1] = load([P, KT * P], f16, wqm_d[1], "wqm1", nc.sync)
            wkm_s[1] = load([P, KT * P], f16, wkm_d[1], "wkm1", nc.gpsimd)
            wkm_s[3] = load([P, KT * P], f16, wkm_d[3], "wkm3", nc.scalar)
            wqm_s[2] = load([P, KT * P], f16, wqm_d[2], "wqm2", nc.sync)
            wkm_s[2] = load([P, KT * P], f16, wkm_d[2], "wkm2", nc.gpsimd)
            wv_c[0] = load([P, 1, C], f16, wvr[:, 0:1, :], "wv0", nc.sync)
            wv_c[1] = load([P, 1, C], f16, wvr[:, 1:2, :], "wv1", nc.gpsimd)
            wv_c[2] = load([P, 1, C], f16, wvr[:, 2:3, :], "wv2", nc.scalar)
            wv_c[3] = load([P, 1, C], f16, wvr[:, 3:4, :], "wv3", nc.gpsimd)
            xh[1][0] = load([P, 1, HT], f16, xTr[:, 0:1, HT:T], "xh10", nc.sync)
            xh[1][1] = load([P, 1, HT], f16, xTr[:, 1:2, HT:T], "xh11", nc.gpsimd)
            xh[1][2] = load([P, 1, HT], f16, xTr[:, 2:3, HT:T], "xh12", nc.sync)
            xh[1][3] = load([P, 1, HT], f16, xTr[:, 3:4, HT:T], "xh13", nc.gpsimd)
            wp_c[0] = load([P, 1, C], f16, wpr[:, 0:1, :], "wp0", nc.sync)
            wp_c[1] = load([P, 1, C], f16, wpr[:, 1:2, :], "wp1", nc.gpsimd)
            wp_c[2] = load([P, 1, C], f16, wpr[:, 2:3, :], "wp2", nc.scalar)
            wp_c[3] = load([P, 1, C], f16, wpr[:, 3:4, :], "wp3", nc.gpsimd)
            bppb_s = load([P, C], f16, bppb, "bppb", nc.sync)

            tri_ap = cm16[:, 0:P]
            ones64_s = cm16[:, P : P + HD]

            qT_t = [persist.tile([P, T], f16, name=f"qT{m}", tag=f"qT{m}") for m in range(MT)]
            kT_t = [persist.tile([P, T], f16, name=f"kT{m}", tag=f"kT{m}") for m in range(MT)]
            v_t = [persist.tile([P, C], f16, name=f"v{r}", tag=f"v{r}") for r in range(RT)]
            yn_t = [persist.tile([P, T], f16, name=f"yn{m}", tag=f"yn{m}") for m in range(MT)]

            # broadcast lower-triangle tile across both heads of an es tile
            tri_b = bass.AP(
                tensor=tri_ap.tensor,
                offset=tri_ap.offset,
                ap=[list(tri_ap.ap[0]), [0, 2], list(tri_ap.ap[1])],
            )

            # ---- emission helpers ----
            def projqk_pass(ms, h, k_on_act):
                pq = {m: pA.tile([P, HT], f32, name=f"pq{m}", tag="pA") for m in ms}
                pk = {m: pB.tile([P, HT], f32, name=f"pk{m}", tag="pB") for m in ms}
                for k in range(KT):
                    rhs = xh[h][k][:, 0, :]
                    for m in ms:
                        nc.tensor.matmul(
                            pq[m], lhsT=wqm_s[m][:, P * k : P * (k + 1)], rhs=rhs,
                            start=(k == 0), stop=(k == KT - 1),
                        )
                        nc.tensor.matmul(
                            pk[m], lhsT=wkm_s[m][:, P * k : P * (k + 1)], rhs=rhs,
                            start=(k == 0), stop=(k == KT - 1),
                        )
                for m in ms:
                    nc.vector.tensor_scalar_add(
                        qT_t[m][:, HT * h : HT * (h + 1)], pq[m],
                        cm32[:, T // P + m : T // P + m + 1],
                    )
                    if k_on_act:
                        nc.scalar.activation(
                            kT_t[m][:, HT * h : HT * (h + 1)], pk[m], AF.Copy
                        )
                    else:
                        nc.vector.tensor_copy(kT_t[m][:, HT * h : HT * (h + 1)], pk[m])

            def proj_v(r):
                h, rr = divmod(r, 4)
                ps = pA.tile([P, C], f32, name="pv", tag="pA")
                for k in range(KT):
                    nc.tensor.matmul(
                        ps,
                        lhsT=xh[h][k][:, 0, P * rr : P * (rr + 1)],
                        rhs=wv_c[k][:, 0, :],
                        start=(k == 0),
                        stop=(k == KT - 1),
                    )
                nc.vector.tensor_copy(v_t[r], ps)

            es_t = {}

            def qk_unit(ic, p, J):
                i0 = max(512 * ic, P * J)
                w = 512 * (ic + 1) - i0
                st = pbig.tile([P, 2, 512], f32, name="st", tag="pbig")
                for h in range(2):
                    nc.tensor.matmul(
                        st[:, h, :w],
                        lhsT=kT_t[p][64 * h : 64 * (h + 1), P * J : P * (J + 1)],
                        rhs=qT_t[p][64 * h : 64 * (h + 1), i0 : i0 + w],
                        start=True,
                        stop=True,
                        tile_position=(64 * h, 0),
                    )
                es = es_pool.tile([P, 2, 512], f16, name="es", tag="es")
                es_t[(ic, p, J)] = es
                nc.scalar.activation(
                    es[:, :, :w], st[:, :, :w], AF.Exp,
                    bias=cm32[:, J : J + 1], scale=0.125,
                )
                if P * J >= 512 * ic:  # diagonal: zero the causal triangle
                    nc.vector.tensor_tensor(
                        out=es[:, :, :P], in0=es[:, :, :P], in1=tri_b, op=ALU.mult
                    )
                return w

            def av_unit(ic, p, J, c0, c1, av, den, first, last):
                i0f = max(512 * ic, P * J)  # es tile origin
                i0 = max(i0f, c0)
                w = c1 - i0
                eo = i0 - i0f
                io = i0 - c0
                es = es_t[(ic, p, J)]
                for h in range(2):
                    nc.tensor.matmul(
                        av[64 * h : 64 * (h + 1), io : io + w],
                        lhsT=v_t[J][:, P * p + 64 * h : P * p + 64 * (h + 1)],
                        rhs=es[:, h, eo : eo + w],
                        start=first,
                        stop=last,
                        tile_position=(0, 64 * h),
                    )
                    nc.tensor.matmul(
                        den[64 * h : 64 * (h + 1), io : io + w],
                        lhsT=ones64_s,
                        rhs=es[:, h, eo : eo + w],
                        start=first,
                        stop=last,
                        tile_position=(0, 64 * h),
                    )
                return w

            def av_fin(p, c0, c1, av, den):
                rden = rden_pool.tile([P, c1 - c0], f32, name="rden", tag="rden")
                nc.vector.reciprocal_approx_fast(out=rden, in_=den)
                nc.vector.tensor_mul(yn_t[p][:, c0:c1], av, rden)

            def po_row(r):
                po = (pA if r % 2 == 0 else pB).tile(
                    [P, C], f32, name=f"po{r}", tag="pA" if r % 2 == 0 else "pB"
                )
                for m in range(MT):
                    nc.tensor.matmul(
                        po, lhsT=yn_t[m][:, P * r : P * (r + 1)], rhs=wp_c[m][:, 0, :],
                        start=(m == 0), stop=(m == MT - 1),
                    )
                finish_evac(r, po)

            def finish_evac(r, po):
                ot = ot_pool.tile([P, C], f16, name="ot", tag="ot")
                nc.vector.tensor_tensor(out=ot, in0=po, in1=bppb_s, op=ALU.add)
                (nc.sync if r % 2 == 0 else nc.gpsimd).dma_start(
                    out=out[P * r : P * (r + 1), :], in_=ot
                )

            # ---- filler stream: (est_pe_ns, closure), dependency-safe order ----
            filler = []

            def add(est, fn):
                filler.append((est, fn))

            # av/po PSUM tiles must be allocated lazily (at closure run time,
            # interleaved with consumption) or the pool rings would deadlock;
            # each phase allocates inside its first closure via a mutable box.
            def add_av_phase(ic, p, c0, c1):
                Js = [
                    J for J in range(4 * (ic + 1))
                    if c1 - max(max(512 * ic, P * J), c0) > 0
                ]
                box = {}

                def unit(J, first, last):
                    if first:
                        box["av"] = pA.tile([P, c1 - c0], f32, name="av", tag="pA")
                        box["den"] = pB.tile([P, c1 - c0], f32, name="den", tag="pB")
                    av_unit(ic, p, J, c0, c1, box["av"], box["den"], first, last)

                for J in Js:
                    w = c1 - max(max(512 * ic, P * J), c0)
                    add(
                        int(0.84 * w + 120),
                        lambda J=J, first=(J == Js[0]), last=(J == Js[-1]): unit(
                            J, first, last
                        ),
                    )
                add(0, lambda: av_fin(p, c0, c1, box["av"], box["den"]))

            def add_B(m):
                # half1 q/k projection for head-pair m, split into two
                # 4-matmul closures (k=0,1 then k=2,3) for finer weave grain
                box = {}

                def part(ks):
                    if ks[0] == 0:
                        box["pq"] = pA.tile([P, HT], f32, name=f"pqB{m}", tag="pA")
                        box["pk"] = pB.tile([P, HT], f32, name=f"pkB{m}", tag="pB")
                    for k in ks:
                        rhs = xh[1][k][:, 0, :]
                        nc.tensor.matmul(
                            box["pq"], lhsT=wqm_s[m][:, P * k : P * (k + 1)], rhs=rhs,
                            start=(k == 0), stop=(k == KT - 1),
                        )
                        nc.tensor.matmul(
                            box["pk"], lhsT=wkm_s[m][:, P * k : P * (k + 1)], rhs=rhs,
                            start=(k == 0), stop=(k == KT - 1),
                        )
                    if ks[-1] == KT - 1:
                        nc.vector.tensor_scalar_add(
                            qT_t[m][:, HT:T], box["pq"],
                            cm32[:, T // P + m : T // P + m + 1],
                        )
                        nc.vector.tensor_copy(kT_t[m][:, HT:T], box["pk"])

                add(900, lambda: part((0, 1)))
                add(900, lambda: part((2, 3)))

            for r in (0, 1, 2, 3):
                add(900, lambda r=r: proj_v(r))
            add_B(0)
            add_B(1)
            for r in (4, 5, 6, 7):
                add(900, lambda r=r: proj_v(r))
            add_B(2)
            add_av_phase(0, 0, 0, 512)
            add_B(3)
            add_av_phase(0, 1, 0, 512)
            add_av_phase(0, 2, 0, 512)
            add_av_phase(0, 3, 0, 512)
            for r in (0, 1, 2, 3):
                add(1000, lambda r=r: po_row(r))
            add_av_phase(1, 0, 512, 1024)
            add_av_phase(1, 1, 512, 1024)
            add_av_phase(1, 2, 512, 1024)

            # ---- emission ----
            projqk_pass((0, 1), 0, True)
            for J in range(4):
                qk_unit(0, 0, J)
            projqk_pass((2, 3), 0, False)

            state = {"pe": 3400.0, "act": 4600.0, "fi": 0}

            def fill(upto):
                while state["fi"] < len(filler) and state["pe"] < upto:
                    est, fn = filler[state["fi"]]
                    fn()
                    state["pe"] += est
                    state["fi"] += 1

            # emit qk units in J-pairs (matches the 2-deep st ring) so the PE
            # stays in row-tiled mode for both before switching back to filler
            for ic, p in ((0, 1), (0, 2), (0, 3), (1, 0), (1, 1), (1, 2), (1, 3)):
                for j0 in range(0, 4 * (ic + 1), 2):
                    for J in (j0, j0 + 1):
                        w = qk_unit(ic, p, J)
                        state["pe"] += 0.42 * w + 80
                        state["act"] += 2.18 * w + 60
                    fill(state["act"] - 400)
            fill(float("inf"))

            # ---- tail: rows 4..7; PSUM for po from the (now idle) st pool ----
            po45 = pbig.tile([P, 2, 512], f32, name="po45", tag="pbig")
            po67 = pbig.tile([P, 2, 512], f32, name="po67", tag="pbig")
            po_tail = {4: po45[:, 0, :], 5: po45[:, 1, :], 6: po67[:, 0, :], 7: po67[:, 1, :]}
            for r in (4, 5, 6, 7):
                for m in range(3):
                    nc.tensor.matmul(
                        po_tail[r], lhsT=yn_t[m][:, P * r : P * (r + 1)],
                        rhs=wp_c[m][:, 0, :], start=(m == 0), stop=False,
                    )

            def tail_half(c0, c1, rows):
                av = pA.tile([P, c1 - c0], f32, name="avT", tag="pA")
                den = pB.tile([P, c1 - c0], f32, name="denT", tag="pB")
                Js = [J for J in range(8) if c1 - max(max(512, P * J), c0) > 0]
                for J in Js:
                    av_unit(1, 3, J, c0, c1, av, den, J == Js[0], J == Js[-1])
                av_fin(3, c0, c1, av, den)
                for r in rows:
                    nc.tensor.matmul(
                        po_tail[r], lhsT=yn_t[3][:, P * r : P * (r + 1)],
                        rhs=wp_c[3][:, 0, :], start=False, stop=True,
                    )
                    finish_evac(r, po_tail[r])

            tail_half(512, 768, (4, 5))
            tail_half(768, 1024, (6, 7))

    nc.compile()
    return nc


def _prep_inputs(x, Wq, bq, Wk, bk, Wv, bv, Wp, bp):
    """Host-side prep: transposes, bias folding, mask tables. Returns in_maps."""
    f16 = np.float16
    wqT = np.ascontiguousarray(Wq.T).astype(f16)
    wkT = np.ascontiguousarray(Wk.T).astype(f16)
    wvT = np.ascontiguousarray(Wv.T).astype(f16)
    wpT = np.ascontiguousarray(Wp.T).astype(f16)

    def mpack(wT, m):  # [P, KT*P]: (p, k*128+j) -> wT[128k+p, 128m+j]
        return np.ascontiguousarray(
            wT.reshape(C // P, P, C)[:, :, P * m : P * (m + 1)].transpose(1, 0, 2)
        ).reshape(P, C)

    bq_pp = np.ascontiguousarray(bq.astype(np.float32).reshape(C // P, P).T)
    # v bias folds into output bias: out = (y' + bv) @ Wp.T + bp
    bpp = (
        Wp.astype(np.float64) @ bv.astype(np.float64) + bp.astype(np.float64)
    ).astype(np.float32)
    bppb = np.broadcast_to(bpp[None, :], (P, C)).astype(f16).copy()
    ones64 = np.ones((P, HD), dtype=f16)
    tri = (np.arange(P)[:, None] <= np.arange(P)[None, :]).astype(f16)  # keep j<=i
    j_idx = np.arange(P)[:, None] + P * np.arange(T // P)[None, :]
    cmask = np.where(j_idx % JD == JD - 1, np.float32(NEG), np.float32(0.0)).astype(
        np.float32
    )

    shared = {
        "wvT": wvT,
        "wpT": wpT,
        "combo16": np.ascontiguousarray(np.concatenate([tri, ones64], axis=1)),
        "combo32": np.ascontiguousarray(
            np.concatenate([cmask, bq_pp], axis=1).astype(np.float32)
        ),
        "bppb": bppb,
    }
    for m in range(C // P):
        shared[f"wqm{m}"] = mpack(wqT, m)
        shared[f"wkm{m}"] = mpack(wkT, m)
    in_maps = []
    for b in range(N_CORES):
        mm = dict(shared)
        mm["xT"] = np.ascontiguousarray(x[b].T).astype(f16)
        in_maps.append(mm)
    return in_maps


def kernel(x, Wq, bq, Wk, bk, Wv, bv, Wp, bp):
    from concourse import bass_utils

    x = np.asarray(x, dtype=np.float32)
    if "nc" not in _CACHE:
        _CACHE["nc"] = _build()
    nc = _CACHE["nc"]
    in_maps = _prep_inputs(
        x,
        np.asarray(Wq, np.float32),
        np.asarray(bq, np.float32),
        np.asarray(Wk, np.float32),
        np.asarray(bk, np.float32),
        np.asarray(Wv, np.float32),
        np.asarray(bv, np.float32),
        np.asarray(Wp, np.float32),
        np.asarray(bp, np.float32),
    )
    res = bass_utils.run_bass_kernel_spmd(nc, in_maps, core_ids=list(range(N_CORES)))
    return np.stack(
        [res.results[b]["out"].astype(np.float32) for b in range(N_CORES)], axis=0
    )


# revision 12
# speedup vs baseline: 1.0146x; 1.0146x over previous
"""Causal self-attention (sparse column mask) on 8 Trainium2 NeuronCores.

Problem: B=8, T=1024, C=512, 8 heads (hd=64).
  q/k/v = x @ W{q,k,v}.T + b;  att = softmax(mask(q k^T / 8));  y = att v
  out = y @ Wp.T + bp
Mask: causal lower-triangle, minus every column j with j % 25 == 24.

Sharding: pure data-parallel over batch — core b computes batch element b.

Per-core kernel design (all matmul operands fp16, PSUM accumulation f32):
  - Host pre-transposes x[b] -> xT [C, T]; q/k weights are packed per
    head-pair m ([P, KT*128], contiguous 1KB DMA lines) so head-pair 0's
    projections complete first and attention starts ~3 us earlier.
  - Inputs stream chunked + prioritized over three DMA queues (sync,
    gpsimd, scalar); the tensor engine issues none so matmuls start ASAP.
  - q/k projections are phase-split by T-half: half0 feeds attention ic=0
    while half1 still streams in. q bias added during PSUM evacuation
    (DVE tensor_scalar); k bias dropped (softmax shift invariance); v bias
    folded into the output bias on host (bp' = Wp @ bv + bp).
  - Attention per query chunk ic (512 wide):
      QK (64x128 row-tiled): per head-pair p, key tile J: two K=64 matmuls
      -> S^T in PSUM; one ACT exp (scale=1/8, per-partition bias -30 on
      j%25==24 columns) -> fp16 SBUF; causal diagonal zeroed by one fp16
      multiply with a broadcast lower-triangle tile on DVE.
      AV (128x64 col-tiled): accumulate y'^T and replicated denominators
      (ones-weight matmuls) over J; rden via approx reciprocal; one
      tensor_tensor multiply PSUM->SBUF fp16.
  - The emission weaves qk J-units (which feed the ACT exp pipeline, the
    per-element bottleneck) with budget-tracked PE filler (v/out/half1
    projections, AV phases) so ACT never starves and PE never idles.
  - fp16 output; the last av phase is column-split so rows 4..7 finish
    pipelined; their PSUM comes from the st pool (idle by then).
"""

import numpy as np

B, T, C = 8, 1024, 512
H = 8
HD = C // H
P = 128
JD = 25  # joined dim; column j masked when j % 25 == 24
N_CORES = 8
NEG = -30.0  # added post-scale; exp(-30) flushes to 0 in fp16

_CACHE = {}


def _build():
    import concourse.bass as bass
    import concourse.mybir as mybir
    import concourse.tile as tile
    from concourse import bacc

    f16 = mybir.dt.float16
    f32 = mybir.dt.float32
    AF = mybir.ActivationFunctionType
    ALU = mybir.AluOpType

    nc = bacc.Bacc("TRN2", target_bir_lowering=False, debug=False)

    KT = C // P  # 4 c_in tiles
    MT = C // P  # 4 c_out tiles (= head pairs)
    RT = T // P  # 8 t tiles
    HT = 512  # half of T

    xT = nc.dram_tensor("xT", [C, T], f16, kind="ExternalInput").ap()
    wqm_d = [
        nc.dram_tensor(f"wqm{m}", [P, KT * P], f16, kind="ExternalInput").ap()
        for m in range(MT)
    ]
    wkm_d = [
        nc.dram_tensor(f"wkm{m}", [P, KT * P], f16, kind="ExternalInput").ap()
        for m in range(MT)
    ]
    wvT = nc.dram_tensor("wvT", [C, C], f16, kind="ExternalInput").ap()
    wpT = nc.dram_tensor("wpT", [C, C], f16, kind="ExternalInput").ap()
    # combo16: tri [:,0:128] | ones64 [:,128:192]; combo32: cmask [:,0:8] | bq [:,8:12]
    combo16 = nc.dram_tensor("combo16", [P, P + HD], f16, kind="ExternalInput").ap()
    combo32 = nc.dram_tensor("combo32", [P, T // P + MT], f32, kind="ExternalInput").ap()
    bppb = nc.dram_tensor("bppb", [P, C], f16, kind="ExternalInput").ap()
    out = nc.dram_tensor("out", [T, C], f16, kind="ExternalOutput").ap()

    with tile.TileContext(nc) as tc:
        with (
            tc.tile_pool(name="const", bufs=1) as const,
            tc.tile_pool(name="persist", bufs=1) as persist,
            tc.tile_pool(name="es", bufs=24) as es_pool,
            tc.tile_pool(name="rden", bufs=4) as rden_pool,
            tc.tile_pool(name="ot", bufs=4) as ot_pool,
            tc.tile_pool(name="pbig", bufs=2, space="PSUM") as pbig,
            tc.tile_pool(name="pA", bufs=2, space="PSUM") as pA,
            tc.tile_pool(name="pB", bufs=2, space="PSUM") as pB,
        ):
            # ---- chunked, prioritized input loads over three DMA queues ----
            def load(shape, dtype, src, tag, eng):
                t = const.tile(shape, dtype, name=tag, tag=tag)
                eng.dma_start(out=t, in_=src)
                return t

            r3 = lambda a: a.rearrange("(a p) n -> p a n", p=P)  # noqa: E731
            xTr, wvr, wpr = map(r3, (xT, wvT, wpT))

            xh = [[None] * KT for _ in range(2)]
            wqm_s, wkm_s = [None] * MT, [None] * MT
            wv_c, wp_c = [None] * KT, [None] * MT
            # strict per-queue priority; sync/gpsimd carry the bulk, scalar
            # (slower queue) takes the small combos + last-pair weights
            # gpsimd's (software-DGE) queue has ~3us cold-start latency, so all
            # phase-A-critical items ride sync/scalar; gpsimd gets late bulk
            cm32 = load([P, T // P + MT], f32, combo32, "c32", nc.scalar)
            cm16 = load([P, P + HD], f16, combo16, "c16", nc.scalar)
            xh[0][0] = load([P, 1, HT], f16, xTr[:, 0:1, 0:HT], "xh00", nc.sync)
            wqm_s[0] = load([P, KT * P], f16, wqm_d[0], "wqm0", nc.sync)
            wkm_s[0] = load([P, KT * P], f16, wkm_d[0], "wkm0", nc.scalar)
            xh[0][1] = load([P, 1, HT], f16, xTr[:, 1:2, 0:HT], "xh01", nc.sync)
            xh[0][2] = load([P, 1, HT], f16, xTr[:, 2:3, 0:HT], "xh02", nc.gpsimd)
            xh[0][3] = load([P, 1, HT], f16, xTr[:, 3:4, 0:HT], "xh03", nc.gpsimd)
            wqm_s[1] = load([P, KT * P], f16, wqm_d[1], "wqm1", nc.sync)
            wkm_s[1] = load([P, KT * P], f16, wkm_d[1], "wkm1", nc.scalar)
            wqm_s[2] = load([P, KT * P], f16, wqm_d[2], "wqm2", nc.gpsimd)
            wkm_s[2] = load([P, KT * P], f16, wkm_d[2], "wkm2", nc.scalar)
            wqm_s[3] = load([P, KT * P], f16, wqm_d[3], "wqm3", nc.sync)
            wkm_s[3] = load([P, KT * P], f16, wkm_d[3], "wkm3", nc.gpsimd)
            wv_c[0] = load([P, 1, C], f16, wvr[:, 0:1, :], "wv0", nc.sync)
            wv_c[1] = load([P, 1, C], f16, wvr[:, 1:2, :], "wv1", nc.gpsimd)
            wv_c[2] = load([P, 1, C], f16, wvr[:, 2:3, :], "wv2", nc.scalar)
            wv_c[3] = load([P, 1, C], f16, wvr[:, 3:4, :], "wv3", nc.gpsimd)
            xh[1][0] = load([P, 1, HT], f16, xTr[:, 0:1, HT:T], "xh10", nc.sync)
            xh[1][1] = load([P, 1, HT], f16, xTr[:, 1:2, HT:T], "xh11", nc.gpsimd)
            xh[1][2] = load([P, 1, HT], f16, xTr[:, 2:3, HT:T], "xh12", nc.sync)
            xh[1][3] = load([P, 1, HT], f16, xTr[:, 3:4, HT:T], "xh13", nc.gpsimd)
            wp_c[0] = load([P, 1, C], f16, wpr[:, 0:1, :], "wp0", nc.sync)
            wp_c[1] = load([P, 1, C], f16, wpr[:, 1:2, :], "wp1", nc.gpsimd)
            wp_c[2] = load([P, 1, C], f16, wpr[:, 2:3, :], "wp2", nc.scalar)
            wp_c[3] = load([P, 1, C], f16, wpr[:, 3:4, :], "wp3", nc.gpsimd)
            bppb_s = load([P, C], f16, bppb, "bppb", nc.sync)

            tri_ap = cm16[:, 0:P]
            ones64_s = cm16[:, P : P + HD]

            qT_t = [persist.tile([P, T], f16, name=f"qT{m}", tag=f"qT{m}") for m in range(MT)]
            kT_t = [persist.tile([P, T], f16, name=f"kT{m}", tag=f"kT{m}") for m in range(MT)]
            v_t = [persist.tile([P, C], f16, name=f"v{r}", tag=f"v{r}") for r in range(RT)]
            yn_t = [persist.tile([P, T], f16, name=f"yn{m}", tag=f"yn{m}") for m in range(MT)]

            # broadcast lower-triangle tile across both heads of an es tile
            tri_b = bass.AP(
                tensor=tri_ap.tensor,
                offset=tri_ap.offset,
                ap=[list(tri_ap.ap[0]), [0, 2], list(tri_ap.ap[1])],
            )

            # ---- emission helpers ----
            def projqk_pass(ms, h, k_on_act):
                pq = {m: pA.tile([P, HT], f32, name=f"pq{m}", tag="pA") for m in ms}
                pk = {m: pB.tile([P, HT], f32, name=f"pk{m}", tag="pB") for m in ms}
                for k in range(KT):
                    rhs = xh[h][k][:, 0, :]
                    for m in ms:
                        nc.tensor.matmul(
                            pq[m], lhsT=wqm_s[m][:, P * k : P * (k + 1)], rhs=rhs,
                            start=(k == 0), stop=(k == KT - 1),
                        )
                        nc.tensor.matmul(
                            pk[m], lhsT=wkm_s[m][:, P * k : P * (k + 1)], rhs=rhs,
                            start=(k == 0), stop=(k == KT - 1),
                        )
                for m in ms:
                    nc.vector.tensor_scalar_add(
                        qT_t[m][:, HT * h : HT * (h + 1)], pq[m],
                        cm32[:, T // P + m : T // P + m + 1],
                    )
                    if k_on_act:
                        nc.scalar.activation(
                            kT_t[m][:, HT * h : HT * (h + 1)], pk[m], AF.Copy
                        )
                    else:
                        nc.vector.tensor_copy(kT_t[m][:, HT * h : HT * (h + 1)], pk[m])

            def proj_v(r):
                h, rr = divmod(r, 4)
                ps = pA.tile([P, C], f32, name="pv", tag="pA")
                for k in range(KT):
                    nc.tensor.matmul(
                        ps,
                        lhsT=xh[h][k][:, 0, P * rr : P * (rr + 1)],
                        rhs=wv_c[k][:, 0, :],
                        start=(k == 0),
                        stop=(k == KT - 1),
                    )
                nc.vector.tensor_copy(v_t[r], ps)

            es_t = {}

            def qk_unit(ic, p, J):
                i0 = max(512 * ic, P * J)
                w = 512 * (ic + 1) - i0
                st = pbig.tile([P, 2, 512], f32, name="st", tag="pbig")
                for h in range(2):
                    nc.tensor.matmul(
                        st[:, h, :w],
                        lhsT=kT_t[p][64 * h : 64 * (h + 1), P * J : P * (J + 1)],
                        rhs=qT_t[p][64 * h : 64 * (h + 1), i0 : i0 + w],
                        start=True,
                        stop=True,
                        tile_position=(64 * h, 0),
                    )
                es = es_pool.tile([P, 2, 512], f16, name="es", tag="es")
                es_t[(ic, p, J)] = es
                nc.scalar.activation(
                    es[:, :, :w], st[:, :, :w], AF.Exp,
                    bias=cm32[:, J : J + 1], scale=0.125,
                )
                if P * J >= 512 * ic:  # diagonal: zero the causal triangle
                    nc.vector.tensor_tensor(
                        out=es[:, :, :P], in0=es[:, :, :P], in1=tri_b, op=ALU.mult
                    )
                return w

            def av_unit(ic, p, J, c0, c1, av, den, first, last):
                i0f = max(512 * ic, P * J)  # es tile origin
                i0 = max(i0f, c0)
                w = c1 - i0
                eo = i0 - i0f
                io = i0 - c0
                es = es_t[(ic, p, J)]
                for h in range(2):
                    nc.tensor.matmul(
                        av[64 * h : 64 * (h + 1), io : io + w],
                        lhsT=v_t[J][:, P * p + 64 * h : P * p + 64 * (h + 1)],
                        rhs=es[:, h, eo : eo + w],
                        start=first,
                        stop=last,
                        tile_position=(0, 64 * h),
                    )
                    nc.tensor.matmul(
                        den[64 * h : 64 * (h + 1), io : io + w],
                        lhsT=ones64_s,
                        rhs=es[:, h, eo : eo + w],
                        start=first,
                        stop=last,
                        tile_position=(0, 64 * h),
                    )
                return w

            def av_fin(p, c0, c1, av, den):
                rden = rden_pool.tile([P, c1 - c0], f32, name="rden", tag="rden")
                nc.vector.reciprocal_approx_fast(out=rden, in_=den)
                nc.vector.tensor_mul(yn_t[p][:, c0:c1], av, rden)

            def po_row(r):
                po = (pA if r % 2 == 0 else pB).tile(
                    [P, C], f32, name=f"po{r}", tag="pA" if r % 2 == 0 else "pB"
                )
                for m in range(MT):
                    nc.tensor.matmul(
                        po, lhsT=yn_t[m][:, P * r : P * (r + 1)], rhs=wp_c[m][:, 0, :],
                        start=(m == 0), stop=(m == MT - 1),
                    )
                finish_evac(r, po)

            def finish_evac(r, po):
                ot = ot_pool.tile([P, C], f16, name="ot", tag="ot")
                nc.vector.tensor_tensor(out=ot, in0=po, in1=bppb_s, op=ALU.add)
                (nc.sync if r % 2 == 0 else nc.gpsimd).dma_start(
                    out=out[P * r : P * (r + 1), :], in_=ot
                )

            # ---- filler stream: (est_pe_ns, closure), dependency-safe order ----
            filler = []

            def add(est, fn):
                filler.append((est, fn))

            # av/po PSUM tiles must be allocated lazily (at closure run time,
            # interleaved with consumption) or the pool rings would deadlock;
            # each phase allocates inside its first closure via a mutable box.
            def add_av_phase(ic, p, c0, c1):
                Js = [
                    J for J in range(4 * (ic + 1))
                    if c1 - max(max(512 * ic, P * J), c0) > 0
                ]
                box = {}

                def unit(J, first, last):
                    if first:
                        box["av"] = pA.tile([P, c1 - c0], f32, name="av", tag="pA")
                        box["den"] = pB.tile([P, c1 - c0], f32, name="den", tag="pB")
                    av_unit(ic, p, J, c0, c1, box["av"], box["den"], first, last)

                for J in Js:
                    w = c1 - max(max(512 * ic, P * J), c0)
                    add(
                        int(0.84 * w + 120),
                        lambda J=J, first=(J == Js[0]), last=(J == Js[-1]): unit(
                            J, first, last
                        ),
                    )
                add(0, lambda: av_fin(p, c0, c1, box["av"], box["den"]))

            def add_B(m):
                # half1 q/k projection for head-pair m, split into two
                # 4-matmul closures (k=0,1 then k=2,3) for finer weave grain
                box = {}

                def part(ks):
                    if ks[0] == 0:
                        box["pq"] = pA.tile([P, HT], f32, name=f"pqB{m}", tag="pA")
                        box["pk"] = pB.tile([P, HT], f32, name=f"pkB{m}", tag="pB")
                    for k in ks:
                        rhs = xh[1][k][:, 0, :]
                        nc.tensor.matmul(
                            box["pq"], lhsT=wqm_s[m][:, P * k : P * (k + 1)], rhs=rhs,
                            start=(k == 0), stop=(k == KT - 1),
                        )
                        nc.tensor.matmul(
                            box["pk"], lhsT=wkm_s[m][:, P * k : P * (k + 1)], rhs=rhs,
                            start=(k == 0), stop=(k == KT - 1),
                        )
                    if ks[-1] == KT - 1:
                        nc.vector.tensor_scalar_add(
                            qT_t[m][:, HT:T], box["pq"],
                            cm32[:, T // P + m : T // P + m + 1],
                        )
                        nc.vector.tensor_copy(kT_t[m][:, HT:T], box["pk"])

                add(900, lambda: part((0, 1)))
                add(900, lambda: part((2, 3)))

            for r in (0, 1, 2, 3):
                add(900, lambda r=r: proj_v(r))
            add_B(0)
            add_B(1)
            for r in (4, 5, 6, 7):
                add(900, lambda r=r: proj_v(r))
            add_B(2)
            add_av_phase(0, 0, 0, 512)
            add_B(3)
            add_av_phase(0, 1, 0, 512)
            add_av_phase(0, 2, 0, 512)
            add_av_phase(0, 3, 0, 512)
            for r in (0, 1, 2, 3):
                add(1000, lambda r=r: po_row(r))
            add_av_phase(1, 0, 512, 1024)
            add_av_phase(1, 1, 512, 1024)
            add_av_phase(1, 2, 512, 1024)

            # ---- emission ----
            # phase A, m-minor, with qk(0,0)/(0,1) J-pairs interleaved so the
            # first exp fires as soon as head-pair 0's projections land
            projqk_pass((0,), 0, True)
            qk_unit(0, 0, 0)
            qk_unit(0, 0, 1)
            projqk_pass((1,), 0, False)
            qk_unit(0, 0, 2)
            qk_unit(0, 0, 3)
            projqk_pass((2,), 0, False)
            qk_unit(0, 1, 0)
            qk_unit(0, 1, 1)
            projqk_pass((3,), 0, False)
            qk_unit(0, 1, 2)
            qk_unit(0, 1, 3)

            state = {"pe": 0.0, "act": 4200.0, "fi": 0}

            def fill(upto):
                while state["fi"] < len(filler) and state["pe"] < upto:
                    est, fn = filler[state["fi"]]
                    fn()
                    state["pe"] += est
                    state["fi"] += 1

            # emit qk units in J-pairs (matches the 2-deep st ring) so the PE
            # stays in row-tiled mode for both before switching back to filler
            for ic, p in ((0, 2), (0, 3), (1, 0), (1, 1), (1, 2), (1, 3)):
                for j0 in range(0, 4 * (ic + 1), 2):
                    for J in (j0, j0 + 1):
                        w = qk_unit(ic, p, J)
                        state["pe"] += 0.42 * w + 80
                        state["act"] += 2.18 * w + 60
                    fill(state["act"] - 400)
            fill(float("inf"))

            # ---- tail: rows 4..7; PSUM for po from the (now idle) st pool ----
            po45 = pbig.tile([P, 2, 512], f32, name="po45", tag="pbig")
            po67 = pbig.tile([P, 2, 512], f32, name="po67", tag="pbig")
            po_tail = {4: po45[:, 0, :], 5: po45[:, 1, :], 6: po67[:, 0, :], 7: po67[:, 1, :]}
            for r in (4, 5, 6, 7):
                for m in range(3):
                    nc.tensor.matmul(
                        po_tail[r], lhsT=yn_t[m][:, P * r : P * (r + 1)],
                        rhs=wp_c[m][:, 0, :], start=(m == 0), stop=False,
                    )

            def tail_half(c0, c1, rows):
                av = pA.tile([P, c1 - c0], f32, name="avT", tag="pA")
                den = pB.tile([P, c1 - c0], f32, name="denT", tag="pB")
                Js = [J for J in range(8) if c1 - max(max(512, P * J), c0) > 0]
                for J in Js:
                    av_unit(1, 3, J, c0, c1, av, den, J == Js[0], J == Js[-1])
                av_fin(3, c0, c1, av, den)
                for r in rows:
                    nc.tensor.matmul(
                        po_tail[r], lhsT=yn_t[3][:, P * r : P * (r + 1)],
                        rhs=wp_c[3][:, 0, :], start=False, stop=True,
                    )
                    finish_evac(r, po_tail[r])

            tail_half(512, 768, (4, 5))
            tail_half(768, 1024, (6, 7))

    nc.compile()
    return nc


def _prep_inputs(x, Wq, bq, Wk, bk, Wv, bv, Wp, bp):
    """Host-side prep: transposes, bias folding, mask tables. Returns in_maps."""
    f16 = np.float16
    wqT = np.ascontiguousarray(Wq.T).astype(f16)
    wkT = np.ascontiguousarray(Wk.T).astype(f16)
    wvT = np.ascontiguousarray(Wv.T).astype(f16)
    wpT = np.ascontiguousarray(Wp.T).astype(f16)

    def mpack(wT, m):  # [P, KT*P]: (p, k*128+j) -> wT[128k+p, 128m+j]
        return np.ascontiguousarray(
            wT.reshape(C // P, P, C)[:, :, P * m : P * (m + 1)].transpose(1, 0, 2)
        ).reshape(P, C)

    bq_pp = np.ascontiguousarray(bq.astype(np.float32).reshape(C // P, P).T)
    # v bias folds into output bias: out = (y' + bv) @ Wp.T + bp
    bpp = (
        Wp.astype(np.float64) @ bv.astype(np.float64) + bp.astype(np.float64)
    ).astype(np.float32)
    bppb = np.broadcast_to(bpp[None, :], (P, C)).astype(f16).copy()
    ones64 = np.ones((P, HD), dtype=f16)
    tri = (np.arange(P)[:, None] <= np.arange(P)[None, :]).astype(f16)  # keep j<=i
    j_idx = np.arange(P)[:, None] + P * np.arange(T // P)[None, :]
    cmask = np.where(j_idx % JD == JD - 1, np.float32(NEG), np.float32(0.0)).astype(
        np.float32
    )

    shared = {
        "wvT": wvT,
        "wpT": wpT,
        "combo16": np.ascontiguousarray(np.concatenate([tri, ones64], axis=1)),
        "combo32": np.ascontiguousarray(
            np.concatenate([cmask, bq_pp], axis=1).astype(np.float32)
        ),
        "bppb": bppb,
    }
    for m in range(C // P):
        shared[f"wqm{m}"] = mpack(wqT, m)
        shared[f"wkm{m}"] = mpack(wkT, m)
    in_maps = []
    for b in range(N_CORES):
        mm = dict(shared)
        mm["xT"] = np.ascontiguousarray(x[b].T).astype(f16)
        in_maps.append(mm)
    return in_maps


def kernel(x, Wq, bq, Wk, bk, Wv, bv, Wp, bp):
    from concourse import bass_utils

    x = np.asarray(x, dtype=np.float32)
    if "nc" not in _CACHE:
        _CACHE["nc"] = _build()
    nc = _CACHE["nc"]
    in_maps = _prep_inputs(
        x,
        np.asarray(Wq, np.float32),
        np.asarray(bq, np.float32),
        np.asarray(Wk, np.float32),
        np.asarray(bk, np.float32),
        np.asarray(Wv, np.float32),
        np.asarray(bv, np.float32),
        np.asarray(Wp, np.float32),
        np.asarray(bp, np.float32),
    )
    res = bass_utils.run_bass_kernel_spmd(nc, in_maps, core_ids=list(range(N_CORES)))
    return np.stack(
        [res.results[b]["out"].astype(np.float32) for b in range(N_CORES)], axis=0
    )
